# revision 27
# baseline (speedup 1.0000x reference)
"""Trainium2 Bass kernel for nn_LocallyDense: gather -> 16 group-GEMMs -> Conv1D(k=32) -> LeakyReLU.

Data-parallel over batch (32 -> 4 per core). Host applies the idx permutation;
stage 1 computes h = x_perm @ W_g per group in bf16.

The k=32 conv is computed with a 2-level Karatsuba decomposition of the tap
dimension: instead of 32 tap-GEMMs per output, 9 sub-correlations of 8 taps
over stride-4 subsampled/differenced sequences (datasets), cutting tensor-
engine work to 9/16 of direct. Derivation (per d-channel, position t):
  level 1: E_s=h[2s+1], P_s=h[2s]-h[2s+1], Q_s=h[2s+2]-h[2s+1]
           A=corr(E,a), B=corr(P,e), C=corr(Q,o) with a=w0+w1 pairs, e/o =
           even/odd taps; y_2u=A_u+B_u, y_2u+1=A_u+C_u
  level 2: the same split applied to each of A,B,C -> 9 corrs of 8 taps.
The 6 needed sums R[i][A2]+R[i][B2], R[i][A2]+R[i][C2] are built in PSUM by
snapshotting the A2 partial (ACT copy psA->psB) and accumulating B2/C2 on
top, so no extra matmul work. Final y phases are DVE adds of PSUM pairs with
the conv bias fused, then LeakyReLU, all in bf16 (rel err ~6e-3 << 2e-2).

DMA transfers are serialized in the timeline model: inputs stream on the SP
queue in execution-priority order; y outputs go out on the Activation queue
to avoid head-of-line blocking. Conv combo weights (2.25x the direct conv
weights) are streamed per (batch-pair, m, corr) and re-fetched for the second
batch pair to bound SBUF.
"""
import numpy as np
import ml_dtypes

import concourse.bass as bass
import concourse.mybir as mybir
import concourse.tile as tile
from concourse.alu_op_type import AluOpType
from concourse import bacc
from concourse.bass_utils import run_bass_kernel_spmd

B, N, F, G, S, D = 32, 1024, 512, 16, 64, 256
KC, O = 32, 512
T = N - KC + 1             # 993
NCORES = 8
BPC = B // NCORES          # 4
NEG_SLOPE = 0.2
F32 = mybir.dt.float32
BF16 = mybir.dt.bfloat16
BF = ml_dtypes.bfloat16

FKT = F // 128             # 4
NP = 1032                  # padded h length
U2 = 249                   # level-2 correlation outputs (4*249 >= 993+3)
NCORR = 9
K2 = 16                    # chunks per corr: 8 taps x 2 d-halves

TRACE = False
_cache = {}


def _sv(t, start, step, n):
    """Strided column view [128, n]: cols start, start+step, ..."""
    return t[:, start:start + step * n].rearrange(
        "p (r f) -> p r f", f=step)[:, :, 0]


def _build():
    nc = bacc.Bacc("TRN2", target_bir_lowering=False, debug=False,
                   num_devices=NCORES)
    xpt_d = nc.dram_tensor("xpt", [BPC, FKT, 128, N], BF16,
                           kind="ExternalInput").ap()
    w_d = nc.dram_tensor("w", [G, 128, FKT, D], BF16,
                         kind="ExternalInput").ap()
    b_d = nc.dram_tensor("b", [G, D], F32, kind="ExternalInput").ap()
    cw_d = nc.dram_tensor("cw", [4, 128, NCORR, K2, 128], BF16,
                          kind="ExternalInput").ap()
    cb_d = nc.dram_tensor("cb", [O], F32, kind="ExternalInput").ap()
    y_d = nc.dram_tensor("y", [BPC, O, T], BF16, kind="ExternalOutput").ap()

    with tile.TileContext(nc) as tc:
        with tc.tile_pool(name="xpt", bufs=2) as p_xpt, \
             tc.tile_pool(name="wg", bufs=G) as p_w, \
             tc.tile_pool(name="ht", bufs=BPC * 2) as p_ht, \
             tc.tile_pool(name="pq", bufs=BPC * 4) as p_pq, \
             tc.tile_pool(name="df", bufs=BPC * 12) as p_df, \
             tc.tile_pool(name="bias", bufs=2) as p_bias, \
             tc.tile_pool(name="cw", bufs=12) as p_cw, \
             tc.tile_pool(name="ss", bufs=16) as p_ss, \
             tc.tile_pool(name="ys", bufs=3) as p_ys, \
             tc.tile_pool(name="yo", bufs=3) as p_yo:

            # ---------------- input DMAs (SP queue, priority order) --------
            xpt_sb = {}

            def fetch_xpt(bb):
                t_ = p_xpt.tile([128, FKT, N], BF16, tag="xpt",
                                name=f"xpt{bb}")
                nc.sync.dma_start(t_[:], xpt_d[bb].rearrange(
                    "kt p n -> p kt n"))
                xpt_sb[bb] = t_

            fetch_xpt(0)
            # conv combo weights streamed in consumption order (A2 sets
            # first), with the first few interleaved into the stage-1 weight
            # stream so stage 2 can start as soon as h(bb0) is done
            CORDER = [0, 3, 6, 1, 4, 7, 2, 5, 8]
            cw_sb = {}

            def fetch_cw(pair, m, crange):
                tiles = cw_sb.setdefault((pair, m), {})
                for c in crange:
                    t_ = p_cw.tile([128, K2, 128], BF16, tag="cw",
                                   name=f"cw{pair}_{m}_{c}")
                    nc.sync.dma_start(t_[:], cw_d[m][:, c])
                    tiles[c] = t_

            w_sb = [p_w.tile([128, FKT, D], BF16, tag="wg", name=f"w{g}")
                    for g in range(G)]
            for g in range(4):
                nc.sync.dma_start(w_sb[g][:], w_d[g])
            b_sb = p_bias.tile([128, G * 2], F32)
            nc.sync.dma_start(b_sb[:], b_d.rearrange("g (m p) -> p (g m)", p=128))
            for g in range(4, G):
                nc.sync.dma_start(w_sb[g][:], w_d[g])
            fetch_xpt(1)
            fetch_cw(0, 0, CORDER[:3])
            fetch_cw(0, 0, CORDER[3:6])
            cb_sb = p_bias.tile([128, 4], F32)
            nc.sync.dma_start(cb_sb[:], cb_d.rearrange("(m p) -> p m", p=128))
            fetch_cw(0, 0, CORDER[6:])
            fetch_xpt(2)
            fetch_xpt(3)
            for m in range(1, 4):
                fetch_cw(0, m, CORDER)
            for m in range(4):
                fetch_cw(1, m, CORDER)

            # ---------------- per-batch state ------------------------------
            ht = {}     # (bb, dh) -> [128, NP] bf16 (padded h)
            pq = {}     # (bb, dh, 0/1) -> P/Q [128, 514]
            dfs = {}    # (bb, dh, i, j) -> diff tiles [128, 256], j in 1,2

            def stage1(bb, p_ps1):
                for dh in range(2):
                    t_ = p_ht.tile([128, NP], BF16, tag="ht",
                                   name=f"ht{bb}_{dh}")
                    ht[(bb, dh)] = t_
                    nc.vector.memset(t_[:, N:NP], 0.0)
                for g in range(G):
                    for dh in range(2):
                        ps_full = p_ps1.tile([128, 256], F32, tag="ps2",
                                             name=f"ps1_{bb}_{g}_{dh}")
                        ps = ps_full[:, :S]
                        for kt in range(FKT):
                            nc.tensor.matmul(
                                ps, w_sb[g][:, kt, dh * 128:(dh + 1) * 128],
                                xpt_sb[bb][:, kt, g * S:(g + 1) * S],
                                start=(kt == 0), stop=(kt == FKT - 1))
                        if dh == 0:
                            nc.scalar.activation(
                                ht[(bb, dh)][:, g * S:(g + 1) * S], ps,
                                mybir.ActivationFunctionType.Identity,
                                bias=b_sb[:, g * 2 + dh: g * 2 + dh + 1])
                        else:
                            nc.vector.tensor_scalar_add(
                                ht[(bb, dh)][:, g * S:(g + 1) * S], ps,
                                b_sb[:, g * 2 + dh: g * 2 + dh + 1])

            def sub(out, a, b_):
                nc.vector.scalar_tensor_tensor(
                    out, a, 1.0, b_, AluOpType.mult, AluOpType.subtract)

            def transforms(bb):
                for dh in range(2):
                    h_ = ht[(bb, dh)]
                    P = p_pq.tile([128, 514], BF16, tag="pq",
                                  name=f"P{bb}_{dh}")
                    Q = p_pq.tile([128, 514], BF16, tag="pq",
                                  name=f"Q{bb}_{dh}")
                    sub(P[:], _sv(h_, 0, 2, 514), _sv(h_, 1, 2, 514))
                    sub(Q[:], _sv(h_, 2, 2, 514), _sv(h_, 1, 2, 514))
                    pq[(bb, dh, 0)] = P
                    pq[(bb, dh, 1)] = Q
                    # level-2 diff datasets per lineage: (i, j=1 (B2), j=2 (C2))
                    for i, src, st0, stp in ((0, h_, 1, 4), (1, P, 0, 2),
                                             (2, Q, 0, 2)):
                        for j, off in ((1, 0), (2, 2 * (stp // 2))):
                            dt_ = p_df.tile([128, 256], BF16, tag="df",
                                            name=f"d{bb}_{dh}_{i}_{j}")
                            if i == 0:
                                va = _sv(src, 1 if j == 1 else 5, 4, 256)
                                vb = _sv(src, 3, 4, 256)
                            else:
                                va = _sv(src, 0 if j == 1 else 2, 2, 256)
                                vb = _sv(src, 1, 2, 256)
                            sub(dt_[:], va, vb)
                            dfs[(bb, dh, i, j)] = dt_

            def rhs(bb, dh, i, j, v2):
                if j == 0:
                    if i == 0:
                        return _sv(ht[(bb, dh)], 3 + 4 * v2, 4, U2)
                    return _sv(pq[(bb, dh, i - 1)], 1 + 2 * v2, 2, U2)
                return dfs[(bb, dh, i, j)][:, v2:v2 + U2]

            def s2_block(pair, m, bb, p_ps2, splits=((0, U2),),
                         ydma=None):
                """9 GEMM-sets for one (batch, m-tile), optionally split into
                column ranges so the drain chain of the final block overlaps
                its own GEMMs. psA_i accumulates A2 then C2; A2 partials are
                snapshotted to SBUF before C2 lands; B2 is its own clean PSUM
                group. Each PSUM bank has a single PE writer; phases are DVE
                ops with at most one PSUM operand, conv bias fused."""
                cwt = cw_sb[(pair, m)]  # dict c -> tile
                y_sb = p_ys.tile([128, 1000], F32, tag="ys")
                y_out = p_yo.tile([128, 1000], BF16, tag="yo")
                cbm = cb_sb[:, m:m + 1]

                def stt(out, a, scal, b_):
                    nc.vector.scalar_tensor_tensor(
                        out, a, scal, b_, AluOpType.add, AluOpType.add)

                for si, (u0, u1) in enumerate(splits):
                    L = u1 - u0
                    sfx = f"{pair}_{m}_{bb}_{si}"
                    psA, psB, sA2 = [], [], []
                    for i in range(3):
                        ps = p_ps2.tile([128, 256], F32, tag="ps2",
                                        name=f"psA{sfx}_{i}")
                        psA.append(ps)
                        c = 3 * i + 0
                        for k2 in range(K2):
                            v2, dh = k2 // 2, k2 % 2
                            nc.tensor.matmul(
                                ps[:, :L], cwt[c][:, k2, :],
                                rhs(bb, dh, i, 0, v2)[:, u0:u1],
                                start=(k2 == 0), stop=False,
                                skip_group_check=True)
                        s_ = p_ss.tile([128, 256], F32, tag="ss",
                                       name=f"sA2_{sfx}_{i}")
                        nc.scalar.activation(s_[:, :L], ps[:, :L],
                                             mybir.ActivationFunctionType.Copy)
                        sA2.append(s_)
                    for i in range(3):
                        pb = p_ps2.tile([128, 256], F32, tag="ps2",
                                        name=f"psB{sfx}_{i}")
                        psB.append(pb)
                        c = 3 * i + 1
                        for k2 in range(K2):
                            v2, dh = k2 // 2, k2 % 2
                            nc.tensor.matmul(
                                pb[:, :L], cwt[c][:, k2, :],
                                rhs(bb, dh, i, 1, v2)[:, u0:u1],
                                start=(k2 == 0), stop=(k2 == K2 - 1))
                    # even phases drain while the C2 GEMMs still run,
                    # freeing the B banks early
                    tAB = p_ss.tile([128, 256], F32, tag="ss",
                                    name=f"tAB{sfx}")
                    tAC = p_ss.tile([128, 256], F32, tag="ss",
                                    name=f"tAC{sfx}")
                    stt(tAB[:, :L], sA2[0][:, :L], 0.0, sA2[1][:, :L])
                    stt(tAC[:, :L], sA2[0][:, :L], 0.0, sA2[2][:, :L])
                    v0 = p_ss.tile([128, 256], F32, tag="ss", name=f"v0_{sfx}")
                    v1 = p_ss.tile([128, 256], F32, tag="ss", name=f"v1_{sfx}")
                    stt(v0[:, :L], tAB[:, :L], cbm, psB[0][:, :L])
                    stt(_sv(y_sb, 4 * u0 + 0, 4, L), v0[:, :L], 0.0,
                        psB[1][:, :L])
                    stt(v1[:, :L], tAC[:, :L], cbm, psB[0][:, :L])
                    stt(_sv(y_sb, 4 * u0 + 1, 4, L), v1[:, :L], 0.0,
                        psB[2][:, :L])
                    for i in range(3):
                        c = 3 * i + 2
                        for k2 in range(K2):
                            v2, dh = k2 // 2, k2 % 2
                            nc.tensor.matmul(
                                psA[i][:, :L], cwt[c][:, k2, :],
                                rhs(bb, dh, i, 2, v2)[:, u0:u1],
                                start=False, stop=(k2 == K2 - 1),
                                skip_group_check=True)
                    sAC = p_ss.tile([128, 256], F32, tag="ss",
                                    name=f"sAC{sfx}")
                    nc.scalar.activation(sAC[:, :L], psA[0][:, :L],
                                         mybir.ActivationFunctionType.Copy)
                    stt(_sv(y_sb, 4 * u0 + 2, 4, L), sAC[:, :L], cbm,
                        psA[1][:, :L])
                    stt(_sv(y_sb, 4 * u0 + 3, 4, L), sAC[:, :L], cbm,
                        psA[2][:, :L])
                    c0, c1 = 4 * u0, min(4 * u1, T)
                    nc.vector.scalar_tensor_tensor(
                        y_out[:, c0:c1], y_sb[:, c0:c1], NEG_SLOPE,
                        y_sb[:, c0:c1], AluOpType.mult, AluOpType.max)
                    (ydma or nc.scalar).dma_start(
                        y_d[bb, m * 128:(m + 1) * 128, c0:c1],
                        y_out[:, c0:c1])

            with tc.tile_pool(name="ps2", bufs=8, space="PSUM") as p_ps2:
                stage1(0, p_ps2)
                transforms(0)
                stage1(1, p_ps2)
                transforms(1)
                s2_block(0, 0, 0, p_ps2)
                s2_block(0, 0, 1, p_ps2)
                stage1(2, p_ps2)
                transforms(2)
                s2_block(0, 1, 0, p_ps2)
                stage1(3, p_ps2)
                transforms(3)
                s2_block(0, 1, 1, p_ps2)
                for m in range(2, 4):
                    s2_block(0, m, 0, p_ps2)
                    s2_block(0, m, 1, p_ps2)
                for m in range(4):
                    s2_block(1, m, 2, p_ps2)
                    if m < 3:
                        s2_block(1, m, 3, p_ps2)
                s2_block(1, 3, 3, p_ps2, splits=((0, 200), (200, U2)),
                         ydma=nc.sync)
    nc.compile()
    return nc


def kernel(x, idx, W, b, conv_w, conv_b):
    x = np.asarray(x); idx = np.asarray(idx); W = np.asarray(W)
    b = np.asarray(b); conv_w = np.asarray(conv_w); conv_b = np.asarray(conv_b)
    if "nc" not in _cache:
        _cache["nc"] = _build()
    nc = _cache["nc"]

    idx_flat = idx.reshape(-1).astype(np.int64)
    xpt = np.ascontiguousarray(
        x[:, idx_flat, :].transpose(0, 2, 1).reshape(B, FKT, 128, N)
    ).astype(BF)
    wg = np.ascontiguousarray(
        W.reshape(G, FKT, 128, D).transpose(0, 2, 1, 3)).astype(BF)

    # 9 combo-weight sets, 8 taps each (Karatsuba level-2 tap combos)
    w4 = conv_w.reshape(8, 4, D, O)     # [v2, r, d, o], tap = 4*v2 + r
    W2 = np.stack([
        w4[:, 0] + w4[:, 1] + w4[:, 2] + w4[:, 3],   # (A,A2)
        w4[:, 0] + w4[:, 1],                         # (A,B2)
        w4[:, 2] + w4[:, 3],                         # (A,C2)
        w4[:, 0] + w4[:, 2],                         # (B,A2)
        w4[:, 0],                                    # (B,B2)
        w4[:, 2],                                    # (B,C2)
        w4[:, 1] + w4[:, 3],                         # (C,A2)
        w4[:, 1],                                    # (C,B2)
        w4[:, 3],                                    # (C,C2)
    ])                                               # [9, 8, D, O]
    # -> cw[m, p, c, k2=(v2,dh), o]
    cw2 = np.ascontiguousarray(
        W2.reshape(NCORR, 8, 2, 128, 4, 128).transpose(4, 3, 0, 1, 2, 5)
        .reshape(4, 128, NCORR, K2, 128)).astype(BF)
    b_c = np.ascontiguousarray(b).astype(np.float32)
    cb_c = np.ascontiguousarray(conv_b).astype(np.float32)

    in_maps = []
    for c in range(NCORES):
        in_maps.append({
            "xpt": xpt[c * BPC:(c + 1) * BPC],
            "w": wg, "b": b_c, "cw": cw2, "cb": cb_c,
        })
    res = run_bass_kernel_spmd(nc, in_maps, core_ids=list(range(NCORES)),
                               trace=TRACE)
    if TRACE and res.exec_time_ns is not None:
        print(f"HW exec time: {res.exec_time_ns} ns")
    y = np.concatenate([r["y"] for r in res.results], axis=0)
    return np.ascontiguousarray(y.transpose(0, 2, 1).astype(np.float32))


# revision 28
# speedup vs baseline: 1.0061x; 1.0061x over previous
"""Trainium2 Bass kernel for nn_LocallyDense: gather -> 16 group-GEMMs -> Conv1D(k=32) -> LeakyReLU.

Data-parallel over batch (32 -> 4 per core). Host applies the idx permutation;
stage 1 computes h = x_perm @ W_g per group in bf16.

The k=32 conv is computed with a 2-level Karatsuba decomposition of the tap
dimension: instead of 32 tap-GEMMs per output, 9 sub-correlations of 8 taps
over stride-4 subsampled/differenced sequences (datasets), cutting tensor-
engine work to 9/16 of direct. Derivation (per d-channel, position t):
  level 1: E_s=h[2s+1], P_s=h[2s]-h[2s+1], Q_s=h[2s+2]-h[2s+1]
           A=corr(E,a), B=corr(P,e), C=corr(Q,o) with a=w0+w1 pairs, e/o =
           even/odd taps; y_2u=A_u+B_u, y_2u+1=A_u+C_u
  level 2: the same split applied to each of A,B,C -> 9 corrs of 8 taps.
The 6 needed sums R[i][A2]+R[i][B2], R[i][A2]+R[i][C2] are built in PSUM by
snapshotting the A2 partial (ACT copy psA->psB) and accumulating B2/C2 on
top, so no extra matmul work. Final y phases are DVE adds of PSUM pairs with
the conv bias fused, then LeakyReLU, all in bf16 (rel err ~6e-3 << 2e-2).

DMA transfers are serialized in the timeline model: inputs stream on the SP
queue in execution-priority order; y outputs go out on the Activation queue
to avoid head-of-line blocking. Conv combo weights (2.25x the direct conv
weights) are streamed per (batch-pair, m, corr) and re-fetched for the second
batch pair to bound SBUF.
"""
import numpy as np
import ml_dtypes

import concourse.bass as bass
import concourse.mybir as mybir
import concourse.tile as tile
from concourse.alu_op_type import AluOpType
from concourse import bacc
from concourse.bass_utils import run_bass_kernel_spmd

B, N, F, G, S, D = 32, 1024, 512, 16, 64, 256
KC, O = 32, 512
T = N - KC + 1             # 993
NCORES = 8
BPC = B // NCORES          # 4
NEG_SLOPE = 0.2
F32 = mybir.dt.float32
BF16 = mybir.dt.bfloat16
BF = ml_dtypes.bfloat16

FKT = F // 128             # 4
NP = 1032                  # padded h length
U2 = 249                   # level-2 correlation outputs (4*249 >= 993+3)
NCORR = 9
K2 = 16                    # chunks per corr: 8 taps x 2 d-halves

TRACE = False
_cache = {}


def _sv(t, start, step, n):
    """Strided column view [128, n]: cols start, start+step, ..."""
    return t[:, start:start + step * n].rearrange(
        "p (r f) -> p r f", f=step)[:, :, 0]


def _build():
    nc = bacc.Bacc("TRN2", target_bir_lowering=False, debug=False,
                   num_devices=NCORES)
    xpt_d = nc.dram_tensor("xpt", [BPC, FKT, 128, N], BF16,
                           kind="ExternalInput").ap()
    w_d = nc.dram_tensor("w", [G, 128, FKT, D], BF16,
                         kind="ExternalInput").ap()
    b_d = nc.dram_tensor("b", [128, G * 2], F32,
                     kind="ExternalInput").ap()
    cw_d = nc.dram_tensor("cw", [4, 128, NCORR, K2, 128], BF16,
                          kind="ExternalInput").ap()
    cb_d = nc.dram_tensor("cb", [128, 4], F32,
                      kind="ExternalInput").ap()
    y_d = nc.dram_tensor("y", [BPC, O, T], BF16, kind="ExternalOutput").ap()

    with tile.TileContext(nc) as tc:
        with tc.tile_pool(name="xpt", bufs=2) as p_xpt, \
             tc.tile_pool(name="wg", bufs=G) as p_w, \
             tc.tile_pool(name="ht", bufs=BPC * 2) as p_ht, \
             tc.tile_pool(name="pq", bufs=BPC * 4) as p_pq, \
             tc.tile_pool(name="df", bufs=BPC * 12) as p_df, \
             tc.tile_pool(name="bias", bufs=2) as p_bias, \
             tc.tile_pool(name="cw", bufs=12) as p_cw, \
             tc.tile_pool(name="ss", bufs=16) as p_ss, \
             tc.tile_pool(name="ys", bufs=3) as p_ys, \
             tc.tile_pool(name="yo", bufs=3) as p_yo:

            # ---------------- input DMAs (SP queue, priority order) --------
            xpt_sb = {}

            def fetch_xpt(bb):
                t_ = p_xpt.tile([128, FKT, N], BF16, tag="xpt",
                                name=f"xpt{bb}")
                nc.sync.dma_start(t_[:], xpt_d[bb].rearrange(
                    "kt p n -> p kt n"))
                xpt_sb[bb] = t_

            fetch_xpt(0)
            # conv combo weights streamed in consumption order (A2 sets
            # first), with the first few interleaved into the stage-1 weight
            # stream so stage 2 can start as soon as h(bb0) is done
            CORDER = [0, 3, 6, 1, 4, 7, 2, 5, 8]
            cw_sb = {}

            def fetch_cw(pair, m, crange):
                tiles = cw_sb.setdefault((pair, m), {})
                for c in crange:
                    t_ = p_cw.tile([128, K2, 128], BF16, tag="cw",
                                   name=f"cw{pair}_{m}_{c}")
                    nc.sync.dma_start(t_[:], cw_d[m][:, c])
                    tiles[c] = t_

            w_sb = [p_w.tile([128, FKT, D], BF16, tag="wg", name=f"w{g}")
                    for g in range(G)]
            for g in range(4):
                nc.sync.dma_start(w_sb[g][:], w_d[g])
            b_sb = p_bias.tile([128, G * 2], F32)
            nc.sync.dma_start(b_sb[:], b_d)
            for g in range(4, G):
                nc.sync.dma_start(w_sb[g][:], w_d[g])
            fetch_xpt(1)
            fetch_cw(0, 0, CORDER[:3])
            fetch_cw(0, 0, CORDER[3:6])
            cb_sb = p_bias.tile([128, 4], F32)
            nc.sync.dma_start(cb_sb[:], cb_d)
            fetch_cw(0, 0, CORDER[6:])
            fetch_xpt(2)
            fetch_xpt(3)
            for m in range(1, 4):
                fetch_cw(0, m, CORDER)
            for m in range(4):
                fetch_cw(1, m, CORDER)

            # ---------------- per-batch state ------------------------------
            ht = {}     # (bb, dh) -> [128, NP] bf16 (padded h)
            pq = {}     # (bb, dh, 0/1) -> P/Q [128, 514]
            dfs = {}    # (bb, dh, i, j) -> diff tiles [128, 256], j in 1,2

            def stage1(bb, p_ps1):
                for dh in range(2):
                    t_ = p_ht.tile([128, NP], BF16, tag="ht",
                                   name=f"ht{bb}_{dh}")
                    ht[(bb, dh)] = t_
                    nc.vector.memset(t_[:, N:NP], 0.0)
                for g in range(G):
                    for dh in range(2):
                        ps_full = p_ps1.tile([128, 256], F32, tag="ps2",
                                             name=f"ps1_{bb}_{g}_{dh}")
                        ps = ps_full[:, :S]
                        for kt in range(FKT):
                            nc.tensor.matmul(
                                ps, w_sb[g][:, kt, dh * 128:(dh + 1) * 128],
                                xpt_sb[bb][:, kt, g * S:(g + 1) * S],
                                start=(kt == 0), stop=(kt == FKT - 1))
                        if dh == 0:
                            nc.scalar.activation(
                                ht[(bb, dh)][:, g * S:(g + 1) * S], ps,
                                mybir.ActivationFunctionType.Identity,
                                bias=b_sb[:, g * 2 + dh: g * 2 + dh + 1])
                        else:
                            nc.vector.tensor_scalar_add(
                                ht[(bb, dh)][:, g * S:(g + 1) * S], ps,
                                b_sb[:, g * 2 + dh: g * 2 + dh + 1])

            def sub(out, a, b_):
                nc.vector.scalar_tensor_tensor(
                    out, a, 1.0, b_, AluOpType.mult, AluOpType.subtract)

            def transforms(bb):
                for dh in range(2):
                    h_ = ht[(bb, dh)]
                    P = p_pq.tile([128, 514], BF16, tag="pq",
                                  name=f"P{bb}_{dh}")
                    Q = p_pq.tile([128, 514], BF16, tag="pq",
                                  name=f"Q{bb}_{dh}")
                    sub(P[:], _sv(h_, 0, 2, 514), _sv(h_, 1, 2, 514))
                    sub(Q[:], _sv(h_, 2, 2, 514), _sv(h_, 1, 2, 514))
                    pq[(bb, dh, 0)] = P
                    pq[(bb, dh, 1)] = Q
                    # level-2 diff datasets per lineage: (i, j=1 (B2), j=2 (C2))
                    for i, src, st0, stp in ((0, h_, 1, 4), (1, P, 0, 2),
                                             (2, Q, 0, 2)):
                        for j, off in ((1, 0), (2, 2 * (stp // 2))):
                            dt_ = p_df.tile([128, 256], BF16, tag="df",
                                            name=f"d{bb}_{dh}_{i}_{j}")
                            if i == 0:
                                va = _sv(src, 1 if j == 1 else 5, 4, 256)
                                vb = _sv(src, 3, 4, 256)
                            else:
                                va = _sv(src, 0 if j == 1 else 2, 2, 256)
                                vb = _sv(src, 1, 2, 256)
                            sub(dt_[:], va, vb)
                            dfs[(bb, dh, i, j)] = dt_

            def rhs(bb, dh, i, j, v2):
                if j == 0:
                    if i == 0:
                        return _sv(ht[(bb, dh)], 3 + 4 * v2, 4, U2)
                    return _sv(pq[(bb, dh, i - 1)], 1 + 2 * v2, 2, U2)
                return dfs[(bb, dh, i, j)][:, v2:v2 + U2]

            def s2_block(pair, m, bb, p_ps2, splits=((0, U2),),
                         ydma=None):
                """9 GEMM-sets for one (batch, m-tile), optionally split into
                column ranges so the drain chain of the final block overlaps
                its own GEMMs. psA_i accumulates A2 then C2; A2 partials are
                snapshotted to SBUF before C2 lands; B2 is its own clean PSUM
                group. Each PSUM bank has a single PE writer; phases are DVE
                ops with at most one PSUM operand, conv bias fused."""
                cwt = cw_sb[(pair, m)]  # dict c -> tile
                y_sb = p_ys.tile([128, 1000], F32, tag="ys")
                y_out = p_yo.tile([128, 1000], BF16, tag="yo")
                cbm = cb_sb[:, m:m + 1]

                def stt(out, a, scal, b_):
                    nc.vector.scalar_tensor_tensor(
                        out, a, scal, b_, AluOpType.add, AluOpType.add)

                for si, (u0, u1) in enumerate(splits):
                    L = u1 - u0
                    sfx = f"{pair}_{m}_{bb}_{si}"
                    psA, psB, sA2 = [], [], []
                    for i in range(3):
                        ps = p_ps2.tile([128, 256], F32, tag="ps2",
                                        name=f"psA{sfx}_{i}")
                        psA.append(ps)
                        c = 3 * i + 0
                        for k2 in range(K2):
                            v2, dh = k2 // 2, k2 % 2
                            nc.tensor.matmul(
                                ps[:, :L], cwt[c][:, k2, :],
                                rhs(bb, dh, i, 0, v2)[:, u0:u1],
                                start=(k2 == 0), stop=False,
                                skip_group_check=True)
                        s_ = p_ss.tile([128, 256], F32, tag="ss",
                                       name=f"sA2_{sfx}_{i}")
                        nc.scalar.activation(s_[:, :L], ps[:, :L],
                                             mybir.ActivationFunctionType.Copy)
                        sA2.append(s_)
                    for i in range(3):
                        pb = p_ps2.tile([128, 256], F32, tag="ps2",
                                        name=f"psB{sfx}_{i}")
                        psB.append(pb)
                        c = 3 * i + 1
                        for k2 in range(K2):
                            v2, dh = k2 // 2, k2 % 2
                            nc.tensor.matmul(
                                pb[:, :L], cwt[c][:, k2, :],
                                rhs(bb, dh, i, 1, v2)[:, u0:u1],
                                start=(k2 == 0), stop=(k2 == K2 - 1))
                    # even phases drain while the C2 GEMMs still run,
                    # freeing the B banks early
                    tAB = p_ss.tile([128, 256], F32, tag="ss",
                                    name=f"tAB{sfx}")
                    tAC = p_ss.tile([128, 256], F32, tag="ss",
                                    name=f"tAC{sfx}")
                    stt(tAB[:, :L], sA2[0][:, :L], 0.0, sA2[1][:, :L])
                    stt(tAC[:, :L], sA2[0][:, :L], 0.0, sA2[2][:, :L])
                    v0 = p_ss.tile([128, 256], F32, tag="ss", name=f"v0_{sfx}")
                    v1 = p_ss.tile([128, 256], F32, tag="ss", name=f"v1_{sfx}")
                    stt(v0[:, :L], tAB[:, :L], cbm, psB[0][:, :L])
                    stt(_sv(y_sb, 4 * u0 + 0, 4, L), v0[:, :L], 0.0,
                        psB[1][:, :L])
                    stt(v1[:, :L], tAC[:, :L], cbm, psB[0][:, :L])
                    stt(_sv(y_sb, 4 * u0 + 1, 4, L), v1[:, :L], 0.0,
                        psB[2][:, :L])
                    for i in range(3):
                        c = 3 * i + 2
                        for k2 in range(K2):
                            v2, dh = k2 // 2, k2 % 2
                            nc.tensor.matmul(
                                psA[i][:, :L], cwt[c][:, k2, :],
                                rhs(bb, dh, i, 2, v2)[:, u0:u1],
                                start=False, stop=(k2 == K2 - 1),
                                skip_group_check=True)
                    sAC = p_ss.tile([128, 256], F32, tag="ss",
                                    name=f"sAC{sfx}")
                    nc.scalar.activation(sAC[:, :L], psA[0][:, :L],
                                         mybir.ActivationFunctionType.Copy)
                    stt(_sv(y_sb, 4 * u0 + 2, 4, L), sAC[:, :L], cbm,
                        psA[1][:, :L])
                    stt(_sv(y_sb, 4 * u0 + 3, 4, L), sAC[:, :L], cbm,
                        psA[2][:, :L])
                    c0, c1 = 4 * u0, min(4 * u1, T)
                    nc.vector.scalar_tensor_tensor(
                        y_out[:, c0:c1], y_sb[:, c0:c1], NEG_SLOPE,
                        y_sb[:, c0:c1], AluOpType.mult, AluOpType.max)
                    (ydma or nc.scalar).dma_start(
                        y_d[bb, m * 128:(m + 1) * 128, c0:c1],
                        y_out[:, c0:c1])

            with tc.tile_pool(name="ps2", bufs=8, space="PSUM") as p_ps2:
                stage1(0, p_ps2)
                transforms(0)
                stage1(1, p_ps2)
                transforms(1)
                s2_block(0, 0, 0, p_ps2)
                s2_block(0, 0, 1, p_ps2)
                stage1(2, p_ps2)
                transforms(2)
                s2_block(0, 1, 0, p_ps2)
                stage1(3, p_ps2)
                transforms(3)
                s2_block(0, 1, 1, p_ps2)
                for m in range(2, 4):
                    s2_block(0, m, 0, p_ps2)
                    s2_block(0, m, 1, p_ps2)
                for m in range(4):
                    s2_block(1, m, 2, p_ps2)
                    if m < 3:
                        s2_block(1, m, 3, p_ps2)
                s2_block(1, 3, 3, p_ps2, splits=((0, 200), (200, U2)),
                         ydma=nc.sync)
    nc.compile()
    return nc


def kernel(x, idx, W, b, conv_w, conv_b):
    x = np.asarray(x); idx = np.asarray(idx); W = np.asarray(W)
    b = np.asarray(b); conv_w = np.asarray(conv_w); conv_b = np.asarray(conv_b)
    if "nc" not in _cache:
        _cache["nc"] = _build()
    nc = _cache["nc"]

    idx_flat = idx.reshape(-1).astype(np.int64)
    xpt = np.ascontiguousarray(
        x[:, idx_flat, :].transpose(0, 2, 1).reshape(B, FKT, 128, N)
    ).astype(BF)
    wg = np.ascontiguousarray(
        W.reshape(G, FKT, 128, D).transpose(0, 2, 1, 3)).astype(BF)

    # 9 combo-weight sets, 8 taps each (Karatsuba level-2 tap combos)
    w4 = conv_w.reshape(8, 4, D, O)     # [v2, r, d, o], tap = 4*v2 + r
    W2 = np.stack([
        w4[:, 0] + w4[:, 1] + w4[:, 2] + w4[:, 3],   # (A,A2)
        w4[:, 0] + w4[:, 1],                         # (A,B2)
        w4[:, 2] + w4[:, 3],                         # (A,C2)
        w4[:, 0] + w4[:, 2],                         # (B,A2)
        w4[:, 0],                                    # (B,B2)
        w4[:, 2],                                    # (B,C2)
        w4[:, 1] + w4[:, 3],                         # (C,A2)
        w4[:, 1],                                    # (C,B2)
        w4[:, 3],                                    # (C,C2)
    ])                                               # [9, 8, D, O]
    # -> cw[m, p, c, k2=(v2,dh), o]
    cw2 = np.ascontiguousarray(
        W2.reshape(NCORR, 8, 2, 128, 4, 128).transpose(4, 3, 0, 1, 2, 5)
        .reshape(4, 128, NCORR, K2, 128)).astype(BF)
    # biases pre-transposed to the on-chip [partition, col] layout so the
    # DMA is contiguous (the [G, D] layout costs 1.8us of scattered reads)
    b_c = np.ascontiguousarray(
        b.reshape(G, 2, 128).transpose(2, 0, 1).reshape(128, G * 2)
    ).astype(np.float32)
    cb_c = np.ascontiguousarray(
        conv_b.reshape(4, 128).T).astype(np.float32)

    in_maps = []
    for c in range(NCORES):
        in_maps.append({
            "xpt": xpt[c * BPC:(c + 1) * BPC],
            "w": wg, "b": b_c, "cw": cw2, "cb": cb_c,
        })
    res = run_bass_kernel_spmd(nc, in_maps, core_ids=list(range(NCORES)),
                               trace=TRACE)
    if TRACE and res.exec_time_ns is not None:
        print(f"HW exec time: {res.exec_time_ns} ns")
    y = np.concatenate([r["y"] for r in res.results], axis=0)
    return np.ascontiguousarray(y.transpose(0, 2, 1).astype(np.float32))


# revision 31
# speedup vs baseline: 1.0164x; 1.0102x over previous
"""Trainium2 Bass kernel for nn_LocallyDense: gather -> 16 group-GEMMs -> Conv1D(k=32) -> LeakyReLU.

Data-parallel over batch (32 -> 4 per core). Host applies the idx permutation;
stage 1 computes h = x_perm @ W_g per group in bf16.

The k=32 conv is computed with a 2-level Karatsuba decomposition of the tap
dimension: instead of 32 tap-GEMMs per output, 9 sub-correlations of 8 taps
over stride-4 subsampled/differenced sequences (datasets), cutting tensor-
engine work to 9/16 of direct. Derivation (per d-channel, position t):
  level 1: E_s=h[2s+1], P_s=h[2s]-h[2s+1], Q_s=h[2s+2]-h[2s+1]
           A=corr(E,a), B=corr(P,e), C=corr(Q,o) with a=w0+w1 pairs, e/o =
           even/odd taps; y_2u=A_u+B_u, y_2u+1=A_u+C_u
  level 2: the same split applied to each of A,B,C -> 9 corrs of 8 taps.
The 6 needed sums R[i][A2]+R[i][B2], R[i][A2]+R[i][C2] are built in PSUM by
snapshotting the A2 partial (ACT copy psA->psB) and accumulating B2/C2 on
top, so no extra matmul work. Final y phases are DVE adds of PSUM pairs with
the conv bias fused, then LeakyReLU, all in bf16 (rel err ~6e-3 << 2e-2).

DMA transfers are serialized in the timeline model: inputs stream on the SP
queue in execution-priority order; y outputs go out on the Activation queue
to avoid head-of-line blocking. Conv combo weights (2.25x the direct conv
weights) are streamed per (batch-pair, m, corr) and re-fetched for the second
batch pair to bound SBUF.
"""
import numpy as np
import ml_dtypes

import concourse.bass as bass
import concourse.mybir as mybir
import concourse.tile as tile
from concourse.alu_op_type import AluOpType
from concourse import bacc
from concourse.bass_utils import run_bass_kernel_spmd

B, N, F, G, S, D = 32, 1024, 512, 16, 64, 256
KC, O = 32, 512
T = N - KC + 1             # 993
NCORES = 8
BPC = B // NCORES          # 4
NEG_SLOPE = 0.2
F32 = mybir.dt.float32
BF16 = mybir.dt.bfloat16
BF = ml_dtypes.bfloat16

FKT = F // 128             # 4
NP = 1032                  # padded h length
U2 = 249                   # level-2 correlation outputs (4*249 >= 993+3)
NCORR = 9
K2 = 16                    # chunks per corr: 8 taps x 2 d-halves

TRACE = False
_cache = {}


def _sv(t, start, step, n):
    """Strided column view [128, n]: cols start, start+step, ..."""
    return t[:, start:start + step * n].rearrange(
        "p (r f) -> p r f", f=step)[:, :, 0]


def _build():
    nc = bacc.Bacc("TRN2", target_bir_lowering=False, debug=False,
                   num_devices=NCORES)
    xpt_d = nc.dram_tensor("xpt", [BPC, FKT, 128, N], BF16,
                           kind="ExternalInput").ap()
    w_d = nc.dram_tensor("w", [G, 128, FKT, D], BF16,
                         kind="ExternalInput").ap()
    b_d = nc.dram_tensor("b", [128, G * 2], F32,
                     kind="ExternalInput").ap()
    cw_d = nc.dram_tensor("cw", [4, 128, NCORR, K2, 128], BF16,
                          kind="ExternalInput").ap()
    cb_d = nc.dram_tensor("cb", [128, 4], F32,
                      kind="ExternalInput").ap()
    y_d = nc.dram_tensor("y", [BPC, O, T], BF16, kind="ExternalOutput").ap()

    with tile.TileContext(nc) as tc:
        with tc.tile_pool(name="xpt", bufs=2) as p_xpt, \
             tc.tile_pool(name="wg", bufs=G) as p_w, \
             tc.tile_pool(name="ht", bufs=BPC * 2) as p_ht, \
             tc.tile_pool(name="pq", bufs=BPC * 4) as p_pq, \
             tc.tile_pool(name="df", bufs=BPC * 12) as p_df, \
             tc.tile_pool(name="bias", bufs=2) as p_bias, \
             tc.tile_pool(name="cw", bufs=12) as p_cw, \
             tc.tile_pool(name="ss", bufs=16) as p_ss, \
             tc.tile_pool(name="ys", bufs=3) as p_ys, \
             tc.tile_pool(name="yo", bufs=3) as p_yo:

            # ---------------- input DMAs (SP queue, priority order) --------
            xpt_sb = {}

            def fetch_xpt(bb):
                t_ = p_xpt.tile([128, FKT, N], BF16, tag="xpt",
                                name=f"xpt{bb}")
                nc.sync.dma_start(t_[:], xpt_d[bb].rearrange(
                    "kt p n -> p kt n"))
                xpt_sb[bb] = t_

            fetch_xpt(0)
            # conv combo weights streamed in consumption order (A2 sets
            # first), with the first few interleaved into the stage-1 weight
            # stream so stage 2 can start as soon as h(bb0) is done
            CORDER = [0, 3, 6, 1, 4, 7, 2, 5, 8]
            cw_sb = {}

            def fetch_cw(pair, m, crange):
                tiles = cw_sb.setdefault((pair, m), {})
                for c in crange:
                    t_ = p_cw.tile([128, K2, 128], BF16, tag="cw",
                                   name=f"cw{pair}_{m}_{c}")
                    nc.sync.dma_start(t_[:], cw_d[m][:, c])
                    tiles[c] = t_

            w_sb = [p_w.tile([128, FKT, D], BF16, tag="wg", name=f"w{g}")
                    for g in range(G)]
            for g in range(4):
                nc.sync.dma_start(w_sb[g][:], w_d[g])
            b_sb = p_bias.tile([128, G * 2], F32)
            nc.sync.dma_start(b_sb[:], b_d)
            for g in range(4, G):
                nc.sync.dma_start(w_sb[g][:], w_d[g])
            fetch_cw(0, 0, CORDER)
            cb_sb = p_bias.tile([128, 4], F32)
            nc.sync.dma_start(cb_sb[:], cb_d)
            fetch_cw(0, 1, CORDER)
            fetch_xpt(1)
            fetch_cw(0, 2, CORDER)
            fetch_cw(0, 3, CORDER)
            fetch_xpt(2)
            fetch_cw(1, 0, CORDER)
            fetch_cw(1, 1, CORDER)
            fetch_xpt(3)
            fetch_cw(1, 2, CORDER)
            fetch_cw(1, 3, CORDER)
            for bb in (2, 3):
                for m in range(4):
                    fetch_cw(bb, m, CORDER)

            # ---------------- per-batch state ------------------------------
            ht = {}     # (bb, dh) -> [128, NP] bf16 (padded h)
            pq = {}     # (bb, dh, 0/1) -> P/Q [128, 514]
            dfs = {}    # (bb, dh, i, j) -> diff tiles [128, 256], j in 1,2

            def stage1(bb, p_ps1):
                for dh in range(2):
                    t_ = p_ht.tile([128, NP], BF16, tag="ht",
                                   name=f"ht{bb}_{dh}")
                    ht[(bb, dh)] = t_
                    nc.vector.memset(t_[:, N:NP], 0.0)
                for g in range(G):
                    for dh in range(2):
                        ps_full = p_ps1.tile([128, 256], F32, tag="ps2",
                                             name=f"ps1_{bb}_{g}_{dh}")
                        ps = ps_full[:, :S]
                        for kt in range(FKT):
                            nc.tensor.matmul(
                                ps, w_sb[g][:, kt, dh * 128:(dh + 1) * 128],
                                xpt_sb[bb][:, kt, g * S:(g + 1) * S],
                                start=(kt == 0), stop=(kt == FKT - 1))
                        if dh == 0:
                            nc.scalar.activation(
                                ht[(bb, dh)][:, g * S:(g + 1) * S], ps,
                                mybir.ActivationFunctionType.Identity,
                                bias=b_sb[:, g * 2 + dh: g * 2 + dh + 1])
                        else:
                            nc.vector.tensor_scalar_add(
                                ht[(bb, dh)][:, g * S:(g + 1) * S], ps,
                                b_sb[:, g * 2 + dh: g * 2 + dh + 1])

            def sub(out, a, b_):
                nc.vector.scalar_tensor_tensor(
                    out, a, 1.0, b_, AluOpType.mult, AluOpType.subtract)

            def transforms(bb):
                for dh in range(2):
                    h_ = ht[(bb, dh)]
                    P = p_pq.tile([128, 514], BF16, tag="pq",
                                  name=f"P{bb}_{dh}")
                    Q = p_pq.tile([128, 514], BF16, tag="pq",
                                  name=f"Q{bb}_{dh}")
                    sub(P[:], _sv(h_, 0, 2, 514), _sv(h_, 1, 2, 514))
                    sub(Q[:], _sv(h_, 2, 2, 514), _sv(h_, 1, 2, 514))
                    pq[(bb, dh, 0)] = P
                    pq[(bb, dh, 1)] = Q
                    # level-2 diff datasets per lineage: (i, j=1 (B2), j=2 (C2))
                    for i, src, st0, stp in ((0, h_, 1, 4), (1, P, 0, 2),
                                             (2, Q, 0, 2)):
                        for j, off in ((1, 0), (2, 2 * (stp // 2))):
                            dt_ = p_df.tile([128, 256], BF16, tag="df",
                                            name=f"d{bb}_{dh}_{i}_{j}")
                            if i == 0:
                                va = _sv(src, 1 if j == 1 else 5, 4, 256)
                                vb = _sv(src, 3, 4, 256)
                            else:
                                va = _sv(src, 0 if j == 1 else 2, 2, 256)
                                vb = _sv(src, 1, 2, 256)
                            sub(dt_[:], va, vb)
                            dfs[(bb, dh, i, j)] = dt_

            def rhs(bb, dh, i, j, v2):
                if j == 0:
                    if i == 0:
                        return _sv(ht[(bb, dh)], 3 + 4 * v2, 4, U2)
                    return _sv(pq[(bb, dh, i - 1)], 1 + 2 * v2, 2, U2)
                return dfs[(bb, dh, i, j)][:, v2:v2 + U2]

            def s2_block(pair, m, bb, p_ps2, splits=((0, U2),),
                         ydma=None):
                """9 GEMM-sets for one (batch, m-tile), optionally split into
                column ranges so the drain chain of the final block overlaps
                its own GEMMs. psA_i accumulates A2 then C2; A2 partials are
                snapshotted to SBUF before C2 lands; B2 is its own clean PSUM
                group. Each PSUM bank has a single PE writer; phases are DVE
                ops with at most one PSUM operand, conv bias fused."""
                cwt = cw_sb[(pair, m)]  # dict c -> tile
                y_sb = p_ys.tile([128, 1000], F32, tag="ys")
                y_out = p_yo.tile([128, 1000], BF16, tag="yo")
                cbm = cb_sb[:, m:m + 1]

                def stt(out, a, scal, b_):
                    nc.vector.scalar_tensor_tensor(
                        out, a, scal, b_, AluOpType.add, AluOpType.add)

                for si, (u0, u1) in enumerate(splits):
                    L = u1 - u0
                    sfx = f"{pair}_{m}_{bb}_{si}"
                    psA, psB, sA2 = [], [], []
                    for i in range(3):
                        ps = p_ps2.tile([128, 256], F32, tag="ps2",
                                        name=f"psA{sfx}_{i}")
                        psA.append(ps)
                        c = 3 * i + 0
                        for k2 in range(K2):
                            v2, dh = k2 // 2, k2 % 2
                            nc.tensor.matmul(
                                ps[:, :L], cwt[c][:, k2, :],
                                rhs(bb, dh, i, 0, v2)[:, u0:u1],
                                start=(k2 == 0), stop=False,
                                skip_group_check=True)
                        s_ = p_ss.tile([128, 256], F32, tag="ss",
                                       name=f"sA2_{sfx}_{i}")
                        nc.scalar.activation(s_[:, :L], ps[:, :L],
                                             mybir.ActivationFunctionType.Copy)
                        sA2.append(s_)
                    for i in range(3):
                        pb = p_ps2.tile([128, 256], F32, tag="ps2",
                                        name=f"psB{sfx}_{i}")
                        psB.append(pb)
                        c = 3 * i + 1
                        for k2 in range(K2):
                            v2, dh = k2 // 2, k2 % 2
                            nc.tensor.matmul(
                                pb[:, :L], cwt[c][:, k2, :],
                                rhs(bb, dh, i, 1, v2)[:, u0:u1],
                                start=(k2 == 0), stop=(k2 == K2 - 1))
                    # even phases drain while the C2 GEMMs still run,
                    # freeing the B banks early
                    tAB = p_ss.tile([128, 256], F32, tag="ss",
                                    name=f"tAB{sfx}")
                    tAC = p_ss.tile([128, 256], F32, tag="ss",
                                    name=f"tAC{sfx}")
                    stt(tAB[:, :L], sA2[0][:, :L], 0.0, sA2[1][:, :L])
                    stt(tAC[:, :L], sA2[0][:, :L], 0.0, sA2[2][:, :L])
                    v0 = p_ss.tile([128, 256], F32, tag="ss", name=f"v0_{sfx}")
                    v1 = p_ss.tile([128, 256], F32, tag="ss", name=f"v1_{sfx}")
                    stt(v0[:, :L], tAB[:, :L], cbm, psB[0][:, :L])
                    stt(_sv(y_sb, 4 * u0 + 0, 4, L), v0[:, :L], 0.0,
                        psB[1][:, :L])
                    stt(v1[:, :L], tAC[:, :L], cbm, psB[0][:, :L])
                    stt(_sv(y_sb, 4 * u0 + 1, 4, L), v1[:, :L], 0.0,
                        psB[2][:, :L])
                    for i in range(3):
                        c = 3 * i + 2
                        for k2 in range(K2):
                            v2, dh = k2 // 2, k2 % 2
                            nc.tensor.matmul(
                                psA[i][:, :L], cwt[c][:, k2, :],
                                rhs(bb, dh, i, 2, v2)[:, u0:u1],
                                start=False, stop=(k2 == K2 - 1),
                                skip_group_check=True)
                    sAC = p_ss.tile([128, 256], F32, tag="ss",
                                    name=f"sAC{sfx}")
                    nc.scalar.activation(sAC[:, :L], psA[0][:, :L],
                                         mybir.ActivationFunctionType.Copy)
                    stt(_sv(y_sb, 4 * u0 + 2, 4, L), sAC[:, :L], cbm,
                        psA[1][:, :L])
                    stt(_sv(y_sb, 4 * u0 + 3, 4, L), sAC[:, :L], cbm,
                        psA[2][:, :L])
                    c0, c1 = 4 * u0, min(4 * u1, T)
                    nc.vector.scalar_tensor_tensor(
                        y_out[:, c0:c1], y_sb[:, c0:c1], NEG_SLOPE,
                        y_sb[:, c0:c1], AluOpType.mult, AluOpType.max)
                    (ydma or nc.scalar).dma_start(
                        y_d[bb, m * 128:(m + 1) * 128, c0:c1],
                        y_out[:, c0:c1])

            with tc.tile_pool(name="ps2", bufs=8, space="PSUM") as p_ps2:
                stage1(0, p_ps2)
                transforms(0)
                s2_block(0, 0, 0, p_ps2)
                s2_block(0, 1, 0, p_ps2)
                s2_block(0, 2, 0, p_ps2)
                stage1(1, p_ps2)
                transforms(1)
                s2_block(0, 3, 0, p_ps2)
                s2_block(1, 0, 1, p_ps2)
                s2_block(1, 1, 1, p_ps2)
                stage1(2, p_ps2)
                transforms(2)
                s2_block(1, 2, 1, p_ps2)
                s2_block(1, 3, 1, p_ps2)
                s2_block(2, 0, 2, p_ps2)
                stage1(3, p_ps2)
                transforms(3)
                s2_block(2, 1, 2, p_ps2)
                s2_block(2, 2, 2, p_ps2)
                s2_block(2, 3, 2, p_ps2)
                for m in range(3):
                    s2_block(3, m, 3, p_ps2)
                s2_block(3, 3, 3, p_ps2, splits=((0, 200), (200, U2)),
                         ydma=nc.sync)
    nc.compile()
    return nc


def kernel(x, idx, W, b, conv_w, conv_b):
    x = np.asarray(x); idx = np.asarray(idx); W = np.asarray(W)
    b = np.asarray(b); conv_w = np.asarray(conv_w); conv_b = np.asarray(conv_b)
    if "nc" not in _cache:
        _cache["nc"] = _build()
    nc = _cache["nc"]

    idx_flat = idx.reshape(-1).astype(np.int64)
    xpt = np.ascontiguousarray(
        x[:, idx_flat, :].transpose(0, 2, 1).reshape(B, FKT, 128, N)
    ).astype(BF)
    wg = np.ascontiguousarray(
        W.reshape(G, FKT, 128, D).transpose(0, 2, 1, 3)).astype(BF)

    # 9 combo-weight sets, 8 taps each (Karatsuba level-2 tap combos)
    w4 = conv_w.reshape(8, 4, D, O)     # [v2, r, d, o], tap = 4*v2 + r
    W2 = np.stack([
        w4[:, 0] + w4[:, 1] + w4[:, 2] + w4[:, 3],   # (A,A2)
        w4[:, 0] + w4[:, 1],                         # (A,B2)
        w4[:, 2] + w4[:, 3],                         # (A,C2)
        w4[:, 0] + w4[:, 2],                         # (B,A2)
        w4[:, 0],                                    # (B,B2)
        w4[:, 2],                                    # (B,C2)
        w4[:, 1] + w4[:, 3],                         # (C,A2)
        w4[:, 1],                                    # (C,B2)
        w4[:, 3],                                    # (C,C2)
    ])                                               # [9, 8, D, O]
    # -> cw[m, p, c, k2=(v2,dh), o]
    cw2 = np.ascontiguousarray(
        W2.reshape(NCORR, 8, 2, 128, 4, 128).transpose(4, 3, 0, 1, 2, 5)
        .reshape(4, 128, NCORR, K2, 128)).astype(BF)
    # biases pre-transposed to the on-chip [partition, col] layout so the
    # DMA is contiguous (the [G, D] layout costs 1.8us of scattered reads)
    b_c = np.ascontiguousarray(
        b.reshape(G, 2, 128).transpose(2, 0, 1).reshape(128, G * 2)
    ).astype(np.float32)
    cb_c = np.ascontiguousarray(
        conv_b.reshape(4, 128).T).astype(np.float32)

    in_maps = []
    for c in range(NCORES):
        in_maps.append({
            "xpt": xpt[c * BPC:(c + 1) * BPC],
            "w": wg, "b": b_c, "cw": cw2, "cb": cb_c,
        })
    res = run_bass_kernel_spmd(nc, in_maps, core_ids=list(range(NCORES)),
                               trace=TRACE)
    if TRACE and res.exec_time_ns is not None:
        print(f"HW exec time: {res.exec_time_ns} ns")
    y = np.concatenate([r["y"] for r in res.results], axis=0)
    return np.ascontiguousarray(y.transpose(0, 2, 1).astype(np.float32))
